# revision 1
# baseline (speedup 1.0000x reference)
"""CrossAttention kernel for 8 Trainium2 NeuronCores.

Sharding: batch (4) x query-row-half (2) -> 8 shards, one per core. Each core
computes the full cross-attention for its 1024 query rows of one batch:
Q/K/V projections, 8 heads of attention, and the output projection. K/V
projections are recomputed by both cores sharing a batch (20% extra flops)
in exchange for zero collectives and a pure-SPMD single NEFF.

Layout trick: x and context are transposed on the host so the contraction
dim (feature dim) lands on SBUF partitions with fast contiguous DMAs; all
device matmuls then run without any on-chip transposes:
  QT = Wq.T @ xT      (i on partitions)     KT = Wk.T @ ctxT
  V  = ctxT.T @ Wv    (natural [nk, i])
  ST_h = KT_h @ QT_h  ([nk, nq], K=64, head pairs packed in PE row groups)
  P = exp(ST * scale) (no max-subtraction; logits are ~N(0,1), safe range)
  O^T_h | den_h = [V_h | ones].T @ P  (denominator rides free in the M dim)
  Y = (O^T/den).T @ Wo + bo

All matmuls run as float32r (fp32 bit layout, reduced-precision multiply,
full-rate 1 cyc/row at free-dim >= 256).
"""

import numpy as np

HEADS = 8
DIM_HEAD = 64
SCALE = DIM_HEAD ** -0.5
B, NQ, DQ = 4, 2048, 512
NK, DC = 1024, 768
INNER = HEADS * DIM_HEAD  # 512
NQH = NQ // 2             # query rows per core
N_CORES = 8
P = 128

_PROG_CACHE = {}


def _build_program():
    import concourse.bacc as bacc
    import concourse.bass as bass
    import concourse.tile as tile
    from concourse import mybir
    from concourse.bass import ts, ds

    f32 = mybir.dt.float32
    f32r = mybir.dt.float32r
    bf16 = mybir.dt.bfloat16
    Exp = mybir.ActivationFunctionType.Exp

    nc = bacc.Bacc(
        "TRN2",
        target_bir_lowering=False,
        debug=False,
        num_devices=N_CORES,
    )

    xT_d = nc.dram_tensor("xT", [DQ, NQH], f32r, kind="ExternalInput")
    ctxT_d = nc.dram_tensor("ctxT", [DC, NK], f32r, kind="ExternalInput")
    Wq_d = nc.dram_tensor("Wq", [DQ, INNER], f32r, kind="ExternalInput")
    Wk_d = nc.dram_tensor("Wk", [DC, INNER], f32r, kind="ExternalInput")
    Wv_d = nc.dram_tensor("Wv", [DC, INNER], f32r, kind="ExternalInput")
    Wo_d = nc.dram_tensor("Wo", [INNER, DQ], f32r, kind="ExternalInput")
    bo_d = nc.dram_tensor("bo", [DQ], f32, kind="ExternalInput")
    ones_d = nc.dram_tensor("ones", [4, 128], bf16, kind="ExternalInput")
    Y_d = nc.dram_tensor("Y", [NQH, DQ], f32, kind="ExternalOutput")

    KQ = DQ // P   # 4  k-tiles for x-side contraction
    KC = DC // P   # 6  k-tiles for context-side contraction
    KI = INNER // P  # 4 k-tiles for inner-dim contraction
    NQT = NQH // P   # 8  query row tiles
    NKT = NK // P    # 8  key row tiles
    NCH = NQH // 512  # 2 nq chunks of 512

    with tile.TileContext(nc) as tc:
        with (
            tc.tile_pool(name="big", bufs=2) as big,
            tc.tile_pool(name="consts", bufs=1) as consts,
            tc.tile_pool(name="ps", bufs=2, space="PSUM") as ps,
            tc.tile_pool(name="rec", bufs=2) as recp,
            tc.tile_pool(name="yp", bufs=2) as yp,
            tc.tile_pool(name="dram", bufs=2, space="DRAM") as dramp,
            tc.tile_pool(name="ep", bufs=8) as ep,
        ):
            # ---- staged inputs: spread DMAs over 4 HWDGE queues ----
            engs = [nc.sync, nc.scalar, nc.gpsimd]
            qi = [0]

            def ld(out_ap, in_ap):
                engs[qi[0] % 3].dma_start(out=out_ap, in_=in_ap)
                qi[0] += 1

            Wk_sb = consts.tile([P, KC, INNER], f32r, tag="wk")
            ld(Wk_sb, Wk_d.ap().rearrange("(ko p) i -> p ko i", p=P))
            ctx_sb = big.tile([P, KC, NK], f32r, tag="big")
            ctx_src = ctxT_d.ap().rearrange("(ko p) n -> p ko n", p=P)
            for k in range(KC):
                ld(ctx_sb[:, k:k + 1, :], ctx_src[:, k:k + 1, :])
            Wq_sb = consts.tile([P, KQ, INNER], f32r, tag="wqo")
            ld(Wq_sb, Wq_d.ap().rearrange("(ko p) i -> p ko i", p=P))
            xT_sb = big.tile([P, KQ, NQH], f32r, tag="big")
            xT_src = xT_d.ap().rearrange("(ko p) n -> p ko n", p=P)
            for k in range(KQ):
                ld(xT_sb[:, k:k + 1, :], xT_src[:, k:k + 1, :])
            Wv_sb = consts.tile([P, KC, INNER], f32r, tag="wv")
            ld(Wv_sb, Wv_d.ap().rearrange("(ko p) i -> p ko i", p=P))
            bo_sb = consts.tile([P, DQ], f32, tag="bo")
            ld(bo_sb, bo_d.ap().unsqueeze(0).to_broadcast((P, DQ)))

            KT_sb = consts.tile([P, KI, NQ // 2], bf16, tag="kt")  # [i, nk] 4x1024
            QT_sb = consts.tile([P, KI, NQH], bf16, tag="qt")      # [i, nq]
            # V in natural [nk, i] layout padded per head to 128 cols:
            # even head h: cols h*128+[0:64]=V_h, [64:128]=ones
            # odd  head h: cols h*128+[0:64]=ones, [64:128]=V_h
            # (ones regions merge to cols 64:192 of every 256-col pair block)
            V_sb = consts.tile([P, NKT, HEADS * P], bf16, tag="v")
            OT_sb = consts.tile([P, KI, NQH], f32r, tag="ot")      # [i, nq]

            ones_src = ones_d.ap().unsqueeze(0).to_broadcast((P, 4, 128))
            for t in range(NKT):
                dv4 = V_sb[:, t, :].rearrange("p (j y) -> p j y", j=4)
                engs[t % 3].dma_start(out=dv4[:, :, 64:192], in_=ones_src)

            # ---- K projection: KT[i, nk] ----
            for m in range(KI):
                for c in range(NK // 512):
                    psk = ps.tile([P, 512], f32, tag="mm")
                    for k in range(KC):
                        nc.tensor.matmul(
                            psk,
                            lhsT=Wk_sb[:, k, ts(m, P)],
                            rhs=ctx_sb[:, k, ds(c * 512, 512)],
                            start=(k == 0),
                            stop=(k == KC - 1),
                        )
                    nc.vector.tensor_copy(KT_sb[:, m, ds(c * 512, 512)], psk)

            # ---- Q projection: QT[i, nq] ----
            for m in range(KI):
                for c in range(NCH):
                    psq = ps.tile([P, 512], f32, tag="mm")
                    for k in range(KQ):
                        nc.tensor.matmul(
                            psq,
                            lhsT=Wq_sb[:, k, ts(m, P)],
                            rhs=xT_sb[:, k, ds(c * 512, 512)],
                            start=(k == 0),
                            stop=(k == KQ - 1),
                        )
                    nc.vector.tensor_copy(QT_sb[:, m, ds(c * 512, 512)], psq)

            # ---- V projection: V[nk, i] scattered into padded head layout ----
            for t in range(NKT):
                psv = ps.tile([P, 512], f32, tag="mm")
                for k in range(KC):
                    nc.tensor.matmul(
                        psv,
                        lhsT=ctx_sb[:, k, ts(t, P)],
                        rhs=Wv_sb[:, k, :],
                        start=(k == 0),
                        stop=(k == KC - 1),
                    )
                pv4 = psv.rearrange("p (j x) -> p j x", j=4)  # x = 128
                dv4 = V_sb[:, t, :].rearrange("p (j y) -> p j y", j=4)  # y = 256
                nc.vector.tensor_copy(dv4[:, :, 0:64], pv4[:, :, 0:64])
                nc.vector.tensor_copy(dv4[:, :, 192:256], pv4[:, :, 64:128])

            # ---- attention, head pairs packed in PE row groups ----
            T_GROUPS = [(0, 3), (3, 3), (6, 2)]
            for j in range(HEADS // 2):
                for c in range(NCH):
                    eA_parts, eB_parts = [], []
                    for t0, tn in T_GROUPS:
                        psA = ps.tile([P, 3, 512], f32, tag="s")
                        psB = ps.tile([P, 3, 512], f32, tag="s")
                        eA_g = ep.tile([P, 3, 512], bf16, tag="e")
                        eB_g = ep.tile([P, 3, 512], bf16, tag="e")
                        eA_parts.append(eA_g)
                        eB_parts.append(eB_g)
                        for i in range(tn):
                            t = t0 + i
                            nc.tensor.matmul(
                                psA[:, i, :],
                                lhsT=KT_sb[0:64, j, ts(t, P)],
                                rhs=QT_sb[0:64, j, ds(c * 512, 512)],
                                start=True,
                                stop=True,
                            )
                            nc.tensor.matmul(
                                psB[:, i, :],
                                lhsT=KT_sb[64:128, j, ts(t, P)],
                                rhs=QT_sb[64:128, j, ds(c * 512, 512)],
                                start=True,
                                stop=True,
                            )
                        nc.scalar.activation(
                            out=eA_g[:, 0:tn, :], in_=psA[:, 0:tn, :],
                            func=Exp, scale=SCALE,
                        )
                        nc.scalar.activation(
                            out=eB_g[:, 0:tn, :], in_=psB[:, 0:tn, :],
                            func=Exp, scale=SCALE,
                        )
                    for h, e_parts in ((2 * j, eA_parts), (2 * j + 1, eB_parts)):
                        po = ps.tile([P, 512], f32, tag="mm")
                        for g, (t0, tn) in enumerate(T_GROUPS):
                            for i in range(tn):
                                t = t0 + i
                                nc.tensor.matmul(
                                    po,
                                    lhsT=V_sb[:, t, ds(h * P, P)],
                                    rhs=e_parts[g][:, i, :],
                                    start=(t == 0),
                                    stop=(t == NKT - 1),
                                )
                        # evict PSUM immediately; normalize off SBUF
                        o_raw = recp.tile([P, 512], f32, tag="oraw")
                        nc.vector.tensor_copy(o_raw, po)
                        olo, ohi = (0, 64) if h % 2 == 0 else (64, 128)
                        dlo = 64 if h % 2 == 0 else 0
                        # chop den row -> [64, 8] for a cheap reciprocal
                        dg = recp.tile([64, 8], f32, tag="dg")
                        nc.gpsimd.dma_start(
                            out=dg, in_=o_raw[dlo:dlo + 1, :]
                        )
                        rg = recp.tile([64, 8], f32, tag="rg")
                        nc.vector.reciprocal(rg, dg)
                        # broadcast 1/den across partitions via DRAM bounce
                        dsc = dramp.tile([512], f32, tag="ds")
                        nc.gpsimd.dma_start(out=dsc, in_=rg)
                        rb = recp.tile([P, 512], f32, tag="rb")
                        nc.gpsimd.dma_start(
                            out=rb[olo:ohi, :],
                            in_=dsc.unsqueeze(0).to_broadcast((64, 512)),
                        )
                        nc.vector.tensor_tensor(
                            OT_sb[olo:ohi, j, ds(c * 512, 512)],
                            o_raw[olo:ohi, :],
                            rb[olo:ohi, :],
                            op=mybir.AluOpType.mult,
                        )

            # ---- output projection: Y = OT.T @ Wo + bo ----
            Wo_sb = consts.tile([P, KI, DQ], f32r, tag="wqo")
            nc.sync.dma_start(
                out=Wo_sb, in_=Wo_d.ap().rearrange("(ko p) i -> p ko i", p=P)
            )
            for m in range(NQT):
                psy = ps.tile([P, 512], f32, tag="mm")
                for k in range(KI):
                    nc.tensor.matmul(
                        psy,
                        lhsT=OT_sb[:, k, ts(m, P)],
                        rhs=Wo_sb[:, k, :],
                        start=(k == 0),
                        stop=(k == KI - 1),
                    )
                y_t = yp.tile([P, DQ], f32, tag="y")
                nc.vector.tensor_tensor(y_t, psy, bo_sb, op=mybir.AluOpType.add)
                nc.sync.dma_start(out=Y_d.ap()[ts(m, P), :], in_=y_t)

    nc.finalize()
    return nc


def _get_program():
    if "nc" not in _PROG_CACHE:
        _PROG_CACHE["nc"] = _build_program()
    return _PROG_CACHE["nc"]


def _ones_bf16():
    import ml_dtypes
    return np.ones((4, 128), dtype=ml_dtypes.bfloat16)


def kernel(x, context, Wq, Wk, Wv, Wo, bo, **_unused):
    from concourse.bass_utils import run_bass_kernel_spmd

    x = np.asarray(x, dtype=np.float32)
    context = np.asarray(context, dtype=np.float32)
    Wq = np.ascontiguousarray(np.asarray(Wq, dtype=np.float32))
    Wk = np.ascontiguousarray(np.asarray(Wk, dtype=np.float32))
    Wv = np.ascontiguousarray(np.asarray(Wv, dtype=np.float32))
    Wo = np.ascontiguousarray(np.asarray(Wo, dtype=np.float32))
    bo = np.ascontiguousarray(np.asarray(bo, dtype=np.float32))

    nc = _get_program()
    in_maps = []
    for core in range(N_CORES):
        b, half = divmod(core, 2)
        xs = np.ascontiguousarray(x[b, half * NQH:(half + 1) * NQH, :].T)
        cs = np.ascontiguousarray(context[b].T)
        in_maps.append(
            {"xT": xs, "ctxT": cs, "Wq": Wq, "Wk": Wk, "Wv": Wv, "Wo": Wo,
             "bo": bo, "ones": _ones_bf16()}
        )

    res = run_bass_kernel_spmd(nc, in_maps, core_ids=list(range(N_CORES)))

    out = np.empty((B, NQ, DQ), np.float32)
    for core in range(N_CORES):
        b, half = divmod(core, 2)
        out[b, half * NQH:(half + 1) * NQH, :] = res.results[core]["Y"]
    return out



# revision 4
# speedup vs baseline: 1.1645x; 1.1645x over previous
"""CrossAttention kernel for 8 Trainium2 NeuronCores.

Sharding: batch (4) x query-row-half (2) -> 8 shards, one per core. Each core
computes the full cross-attention for its 1024 query rows of one batch:
Q/K/V projections, 8 heads of attention, and the output projection. K/V
projections are recomputed by both cores sharing a batch (20% extra flops)
in exchange for zero collectives and a pure-SPMD single NEFF.

All matmul inputs are bf16 (1 cyc/row on the PE, half the HBM traffic of
fp32; verified final rel-err ~5e-3 vs the 2e-2 budget). Layouts put the
contraction dim on SBUF partitions so no on-chip transposes are needed:
  KT = Wk.T @ ctxT      QT = Wq.T @ xT      V = ctxT.T @ Wv
  S_h = K_h Q_h^T       (64-row PE tiles T0/T8, head pairs packed)
  P = exp(S * scale)    (Scalar engine; no max-subtraction, logits ~N(0,1))
  O^T_h | den_h = [V_h | ones].T @ P   (denominator rides in the M dim)
  Y = (O^T/den).T @ Wo + bo

Schedule: the Scalar engine's exp throughput (~8.2us per head-pair chunk)
exceeds the PE's score-matmul time (~3.4us), so projection and PV matmuls
are interleaved between score groups as fillers to keep the PE busy while
exp catches up. PSUM budget: score groups 2x(2 banks) + proj 2 + PV
accumulators 2 = 8 banks.
"""

import numpy as np

HEADS = 8
DIM_HEAD = 64
SCALE = DIM_HEAD ** -0.5
B, NQ, DQ = 4, 2048, 512
NK, DC = 4 * 256, 768
INNER = HEADS * DIM_HEAD  # 512
NQH = NQ // 2             # query rows per core
N_CORES = 8
P = 128

_PROG_CACHE = {}


def _build_program():
    import concourse.bacc as bacc
    import concourse.tile as tile
    from concourse import mybir
    from concourse.bass import ts, ds

    f32 = mybir.dt.float32
    bf16 = mybir.dt.bfloat16
    Exp = mybir.ActivationFunctionType.Exp

    nc = bacc.Bacc(
        "TRN2",
        target_bir_lowering=False,
        debug=False,
        num_devices=N_CORES,
    )

    xT_d = nc.dram_tensor("xT", [DQ, NQH], bf16, kind="ExternalInput")
    ctxT_d = nc.dram_tensor("ctxT", [DC, NK], bf16, kind="ExternalInput")
    Wq_d = nc.dram_tensor("Wq", [DQ, INNER], bf16, kind="ExternalInput")
    Wk_d = nc.dram_tensor("Wk", [DC, INNER], bf16, kind="ExternalInput")
    Wv_d = nc.dram_tensor("Wv", [DC, INNER], bf16, kind="ExternalInput")
    Wo_d = nc.dram_tensor("Wo", [INNER, DQ], bf16, kind="ExternalInput")
    bo_d = nc.dram_tensor("bo", [DQ], f32, kind="ExternalInput")
    ones_d = nc.dram_tensor("ones", [4, 128], bf16, kind="ExternalInput")
    Y_d = nc.dram_tensor("Y", [NQH, DQ], f32, kind="ExternalOutput")

    KQ = DQ // P      # 4  k-tiles for x-side contraction
    KC = DC // P      # 6  k-tiles for context-side contraction
    KI = INNER // P   # 4  k-tiles for inner-dim contraction (= head pairs)
    NKT = NK // P     # 8  key row tiles
    NCH = NQH // 512  # 2  nq chunks of 512

    with tile.TileContext(nc) as tc:
        with (
            tc.tile_pool(name="consts", bufs=1) as consts,
            tc.tile_pool(name="st", bufs=2, space="PSUM") as stp,
            tc.tile_pool(name="mm", bufs=2, space="PSUM") as mmp,
            tc.tile_pool(name="po", bufs=2, space="PSUM") as pop,
            tc.tile_pool(name="ep", bufs=10) as ep,
            tc.tile_pool(name="rec", bufs=2) as recp,
            tc.tile_pool(name="yp", bufs=2) as yp,
            tc.tile_pool(name="dram", bufs=2, space="DRAM") as dramp,
        ):
            # ---- staged inputs: gpsimd issues DMAs for 25ns each (sync for
            # a few); scalar/vector stay clean for exp/evictions ----
            engs = [nc.gpsimd, nc.sync]
            qi = [0]

            def ld(out_ap, in_ap):
                engs[qi[0] % 2].dma_start(out=out_ap, in_=in_ap)
                qi[0] += 1

            Wk_sb = consts.tile([P, KC, INNER], bf16, tag="wk")
            ld(Wk_sb, Wk_d.ap().rearrange("(ko p) i -> p ko i", p=P))
            ctx_sb = consts.tile([P, KC, NK], bf16, tag="ctx")
            ctx_src = ctxT_d.ap().rearrange("(ko p) n -> p ko n", p=P)
            for c in range(2):
                for k in range(KC):
                    ld(ctx_sb[:, k:k + 1, ds(c * 512, 512)],
                       ctx_src[:, k:k + 1, ds(c * 512, 512)])
            Wq_sb = consts.tile([P, KQ, INNER], bf16, tag="wq")
            ld(Wq_sb, Wq_d.ap().rearrange("(ko p) i -> p ko i", p=P))
            xT_sb = consts.tile([P, KQ, NQH], bf16, tag="x")
            xT_src = xT_d.ap().rearrange("(ko p) n -> p ko n", p=P)
            for c in range(NCH):
                for k in range(KQ):
                    ld(xT_sb[:, k:k + 1, ds(c * 512, 512)],
                       xT_src[:, k:k + 1, ds(c * 512, 512)])
            Wv_sb = consts.tile([P, KC, INNER], bf16, tag="wv")
            ld(Wv_sb, Wv_d.ap().rearrange("(ko p) i -> p ko i", p=P))
            # V in natural [nk, i] layout padded per head to 128 cols:
            # even head h: cols h*128+[0:64]=V_h, [64:128]=ones
            # odd  head h: cols h*128+[0:64]=ones, [64:128]=V_h
            # (ones regions merge to cols 64:192 of every 256-col pair block)
            V_sb = consts.tile([P, NKT, HEADS * P], bf16, tag="v")
            ones_src = ones_d.ap().unsqueeze(0).to_broadcast((P, 4, 128))
            for t in range(NKT):
                dv4 = V_sb[:, t, :].rearrange("p (j y) -> p j y", j=4)
                ld(dv4[:, :, 64:192], ones_src)
            Wo_sb = consts.tile([P, KI, DQ], bf16, tag="wo")
            ld(Wo_sb, Wo_d.ap().rearrange("(ko p) i -> p ko i", p=P))
            bo_sb = consts.tile([P, DQ], f32, tag="bo")
            ld(bo_sb, bo_d.ap().unsqueeze(0).to_broadcast((P, DQ)))

            KT_sb = consts.tile([P, KI, NK], bf16, tag="kt")   # [i, nk]
            QT_sb = consts.tile([P, KI, NQH], bf16, tag="qt")  # [i, nq]
            OT_sb = consts.tile([P, KI, NQH], bf16, tag="ot")  # [i, nq] norm'd

            # ---- emission helpers; each emits PE work plus its evictions ----
            def emit_kproj(j, c):
                psk = mmp.tile([P, 512], f32, tag="mm")
                for k in range(KC):
                    nc.tensor.matmul(
                        psk, lhsT=Wk_sb[:, k, ts(j, P)],
                        rhs=ctx_sb[:, k, ds(c * 512, 512)],
                        start=(k == 0), stop=(k == KC - 1),
                    )
                nc.vector.tensor_copy(KT_sb[:, j, ds(c * 512, 512)], psk)

            def emit_qproj(j, c):
                psq = mmp.tile([P, 512], f32, tag="mm")
                for k in range(KQ):
                    nc.tensor.matmul(
                        psq, lhsT=Wq_sb[:, k, ts(j, P)],
                        rhs=xT_sb[:, k, ds(c * 512, 512)],
                        start=(k == 0), stop=(k == KQ - 1),
                    )
                nc.vector.tensor_copy(QT_sb[:, j, ds(c * 512, 512)], psq)

            def emit_vproj(t):
                psv = mmp.tile([P, 512], f32, tag="mm")
                for k in range(KC):
                    nc.tensor.matmul(
                        psv, lhsT=ctx_sb[:, k, ts(t, P)], rhs=Wv_sb[:, k, :],
                        start=(k == 0), stop=(k == KC - 1),
                    )
                pv4 = psv.rearrange("p (j x) -> p j x", j=4)
                dv4 = V_sb[:, t, :].rearrange("p (j y) -> p j y", j=4)
                nc.vector.tensor_copy(dv4[:, :, 0:64], pv4[:, :, 0:64])
                nc.vector.tensor_copy(dv4[:, :, 192:256], pv4[:, :, 64:128])

            # score group g of (j, c): t-tiles {2g, 2g+1} for both heads of
            # pair j. A = head 2j (KT/QT rows 0:64, PE tile T0); B = head
            # 2j+1 (rows 64:128, tile T8). Adjacent T0/T8 matmuls can run
            # concurrently on disjoint PE row-halves.
            e_tiles = {}

            def emit_st_group(j, c, g):
                psA = stp.tile([P, 2, 512], f32, tag="st")
                psB = stp.tile([P, 2, 512], f32, tag="st")
                for i, t in enumerate((2 * g, 2 * g + 1)):
                    nc.tensor.matmul(
                        psA[:, i, :], lhsT=KT_sb[0:64, j, ts(t, P)],
                        rhs=QT_sb[0:64, j, ds(c * 512, 512)],
                        start=True, stop=True,
                    )
                    nc.tensor.matmul(
                        psB[:, i, :], lhsT=KT_sb[64:128, j, ts(t, P)],
                        rhs=QT_sb[64:128, j, ds(c * 512, 512)],
                        start=True, stop=True,
                    )
                eA = ep.tile([P, 2, 512], bf16, tag="e")
                eB = ep.tile([P, 2, 512], bf16, tag="e")
                nc.scalar.activation(out=eA, in_=psA, func=Exp, scale=SCALE)
                nc.scalar.activation(out=eB, in_=psB, func=Exp, scale=SCALE)
                e_tiles[(j, c, g, 0)] = eA
                e_tiles[(j, c, g, 1)] = eB

            # PV filler block g for (j, c): accumulate t-tiles {2g, 2g+1}
            # into both heads' accumulators. g==0 allocates, g==3 evicts +
            # normalizes.
            po_tiles = {}

            def emit_pv_group(j, c, g):
                if g == 0:
                    po_tiles[0] = pop.tile([P, 512], f32, tag="po",
                                           name=f"po_{j}_{c}_a")
                    po_tiles[1] = pop.tile([P, 512], f32, tag="po",
                                           name=f"po_{j}_{c}_b")
                for ab in range(2):
                    h = 2 * j + ab
                    po = po_tiles[ab]
                    e = e_tiles.pop((j, c, g, ab))
                    for i, t in enumerate((2 * g, 2 * g + 1)):
                        nc.tensor.matmul(
                            po, lhsT=V_sb[:, t, ds(h * P, P)], rhs=e[:, i, :],
                            start=(t == 0), stop=(t == NKT - 1),
                        )
                if g == 3:
                    for ab in range(2):
                        _norm_head(j, c, 2 * j + ab, po_tiles[ab])

            def _norm_head(j, c, h, po):
                # evict PSUM fast; normalize off SBUF. den rides replicated
                # on the ones-partitions; broadcast 1/den via a DRAM bounce.
                o_raw = recp.tile([P, 512], f32, tag="oraw")
                nc.vector.tensor_copy(o_raw, po)
                olo, ohi = (0, 64) if h % 2 == 0 else (64, 128)
                dlo = 64 if h % 2 == 0 else 0
                dg = recp.tile([64, 8], f32, tag="dg")
                nc.gpsimd.dma_start(out=dg, in_=o_raw[dlo:dlo + 1, :])
                rg = recp.tile([64, 8], f32, tag="rg")
                nc.vector.reciprocal(rg, dg)
                dsc = dramp.tile([512], f32, tag="ds")
                nc.gpsimd.dma_start(out=dsc, in_=rg)
                rb = recp.tile([P, 512], f32, tag="rb")
                nc.gpsimd.dma_start(
                    out=rb[olo:ohi, :],
                    in_=dsc.unsqueeze(0).to_broadcast((64, 512)),
                )
                nc.vector.tensor_tensor(
                    OT_sb[olo:ohi, j, ds(c * 512, 512)],
                    o_raw[olo:ohi, :], rb[olo:ohi, :],
                    op=mybir.AluOpType.mult,
                )

            def emit_yproj(m):
                psy = mmp.tile([P, 512], f32, tag="mm")
                for k in range(KI):
                    nc.tensor.matmul(
                        psy, lhsT=OT_sb[:, k, ts(m, P)], rhs=Wo_sb[:, k, :],
                        start=(k == 0), stop=(k == KI - 1),
                    )
                y_t = yp.tile([P, DQ], f32, tag="y")
                nc.vector.tensor_tensor(y_t, psy, bo_sb,
                                        op=mybir.AluOpType.add)
                nc.sync.dma_start(out=Y_d.ap()[ts(m, P), :], in_=y_t)

            # ---- schedule ----
            # filler queue: closures emitting one PE block each, popped
            # between score groups so the PE stays busy while exp runs.
            from collections import deque
            fillers = deque()

            def pop_fillers(n):
                for _ in range(n):
                    if fillers:
                        fillers.popleft()()

            emit_kproj(0, 0)
            emit_kproj(0, 1)
            emit_qproj(0, 0)
            emit_qproj(0, 1)

            # j=0: fillers are the V projection (PV needs all of V, so no PV
            # fillers yet); j>=1: fillers are PV of the previous pair.
            for t in range(NKT):
                fillers.append(lambda t=t: emit_vproj(t))

            for j in range(KI):
                for c in range(NCH):
                    for g in range(4):
                        emit_st_group(j, c, g)
                        pop_fillers(1 if j == 0 else 2)
                # between (j,c0) and (j,c1) nothing extra; after (j,c1):
                if j == 0:
                    # drain remaining V proj, then next projections
                    pop_fillers(len(fillers))
                if j < KI - 1:
                    emit_kproj(j + 1, 0)
                    emit_kproj(j + 1, 1)
                    emit_qproj(j + 1, 0)
                    emit_qproj(j + 1, 1)
                # queue PV of this pair as fillers for the next pair
                for c in range(NCH):
                    for g in range(4):
                        fillers.append(
                            lambda j=j, c=c, g=g: emit_pv_group(j, c, g))

            # tail: PV of the last pair, then the output projection.
            pop_fillers(len(fillers))
            for c in range(NCH):
                for m in range(4):
                    emit_yproj(c * 4 + m)

    nc.finalize()
    return nc


def _get_program():
    if "nc" not in _PROG_CACHE:
        _PROG_CACHE["nc"] = _build_program()
    return _PROG_CACHE["nc"]


def _bf16(a):
    import ml_dtypes
    return np.ascontiguousarray(a).astype(ml_dtypes.bfloat16)


def _ones_bf16():
    import ml_dtypes
    return np.ones((4, 128), dtype=ml_dtypes.bfloat16)


def _build_in_maps(x, context, Wq, Wk, Wv, Wo, bo):
    x = np.asarray(x, dtype=np.float32)
    context = np.asarray(context, dtype=np.float32)
    Wq_b = _bf16(np.asarray(Wq, dtype=np.float32))
    Wk_b = _bf16(np.asarray(Wk, dtype=np.float32))
    Wv_b = _bf16(np.asarray(Wv, dtype=np.float32))
    Wo_b = _bf16(np.asarray(Wo, dtype=np.float32))
    bo_f = np.ascontiguousarray(np.asarray(bo, dtype=np.float32))
    ones = _ones_bf16()
    in_maps = []
    for core in range(N_CORES):
        b, half = divmod(core, 2)
        xs = _bf16(x[b, half * NQH:(half + 1) * NQH, :].T)
        cs = _bf16(context[b].T)
        in_maps.append(
            {"xT": xs, "ctxT": cs, "Wq": Wq_b, "Wk": Wk_b, "Wv": Wv_b,
             "Wo": Wo_b, "bo": bo_f, "ones": ones}
        )
    return in_maps


def kernel(x, context, Wq, Wk, Wv, Wo, bo, **_unused):
    from concourse.bass_utils import run_bass_kernel_spmd

    nc = _get_program()
    in_maps = _build_in_maps(x, context, Wq, Wk, Wv, Wo, bo)
    res = run_bass_kernel_spmd(nc, in_maps, core_ids=list(range(N_CORES)))

    out = np.empty((B, NQ, DQ), np.float32)
    for core in range(N_CORES):
        b, half = divmod(core, 2)
        out[b, half * NQH:(half + 1) * NQH, :] = res.results[core]["Y"]
    return out
